# revision 1
# baseline (speedup 1.0000x reference)
"""Trainium2 Bass kernel for nn_CrossAttention (3x3 scale-grid cross attention).

Reference computation (per batch b):
    WV_i = V_i @ W.T + b                    (video projection, i in 0..2)
    S_ij = (WV_i @ A_j.T) / sqrt(C)         [T, S] scores
    P_ij = softmax(S_ij, axis=-1)
    fv[i,j] = P_ij @ A_j        -> out[0, i, j, b]
    fa[j,i] = P_ij.T @ V_i      -> out[1, j, i, b]

Sharding: data-parallel over batch B=8 across the 8 NeuronCores (one batch
element per core). W/b replicated. Each core runs all 9 (i,j) attention pairs
for its batch element.

On-chip plan (per core): bf16 matmul paths with fp32 PSUM accumulation; the
PE runs a pure stream of N=512 matmuls (scores -> fa -> fv per pair).
P is normalized in-place right after the exp, so both outputs are plain
PSUM->SBUF copies. P^T (fv's stationary operand) is produced by DMA xbar
transposes of P in two halves, each issued as soon as its blocks are done,
so the xbar runs under the tail of scores / the fa matmuls — no PE
transposes at all. Each pair's 8 fa (resp. fv) output blocks are staged
into one SBUF tile and shipped with a single 2 MB DMA.
"""

import numpy as np
from contextlib import ExitStack

import ml_dtypes

import concourse.bacc as bacc
import concourse.bass as bass
import concourse.mybir as mybir
import concourse.tile as tile
from concourse.bass_utils import run_bass_kernel_spmd

BF16 = mybir.dt.bfloat16
F32 = mybir.dt.float32
AF = mybir.ActivationFunctionType

B, T, C = 8, 1024, 512
P = 128
NT = T // P   # 8 row blocks
ND = C // P   # 4 feature chunks
SCALE = 1.0 / float(np.sqrt(C))

_CACHE = {}

# timing-diagnostic variants (wrong results except "full"):
#   "full"      - the real kernel
#   "no_xpose"  - skip the DMA xbar transposes (fv reads stale P^T)
#   "mm_floor"  - PE matmul stream only: no exp/normalize/transposes/
#                 copies/output DMAs (measures the pure-MM floor)
_VARIANT = "full"
# scores loop order: True = dc outer / h inner (consecutive matmuls share
# the stationary operand -> half the LDWEIGHTS, but alternate PSUM banks)
_SCORES_SHARE_LDW = False
# one merged 2MB output DMA per (pair, output) instead of 8x 256KB
_MERGED_OUT_DMA = False
# software-pipelined emission [scores_n | fv_{n-1} | fa_n] vs plain
# [scores_n | fa_n | fv_n]
_PIPELINED = True


def _build(repeat=1, loop=0):
    key = ("nc", repeat, loop, _VARIANT, _SCORES_SHARE_LDW, _MERGED_OUT_DMA,
           _PIPELINED)
    if key in _CACHE:
        return _CACHE[key]

    nc = bacc.Bacc("TRN2", target_bir_lowering=False, debug=False, num_devices=8)

    a_dram = [nc.dram_tensor(f"a{j}", [T, C], BF16, kind="ExternalInput").ap()
              for j in range(3)]
    v_dram = [nc.dram_tensor(f"v{i}", [T, C], BF16, kind="ExternalInput").ap()
              for i in range(3)]
    at_dram = [nc.dram_tensor(f"at{j}", [C, T], BF16, kind="ExternalInput").ap()
               for j in range(3)]
    vt_dram = [nc.dram_tensor(f"vt{i}", [C, T], BF16, kind="ExternalInput").ap()
               for i in range(3)]
    wt_dram = nc.dram_tensor("WT", [C, C], BF16, kind="ExternalInput").ap()
    b_dram = nc.dram_tensor("bvec", [ND, P, 1], F32, kind="ExternalInput").ap()
    out_dram = nc.dram_tensor("out", [2, 3, 3, T, C], F32, kind="ExternalOutput").ap()

    with ExitStack() as ctx:
        tc = ctx.enter_context(tile.TileContext(nc))

        const = ctx.enter_context(tc.tile_pool(name="const", bufs=1))
        big = ctx.enter_context(tc.tile_pool(name="big", bufs=1))
        work = ctx.enter_context(tc.tile_pool(name="work", bufs=1))
        small = ctx.enter_context(tc.tile_pool(name="small", bufs=1))
        nstage = 2 if _MERGED_OUT_DMA else 16
        stage = ctx.enter_context(tc.tile_pool(name="stage", bufs=nstage))

        ps_s = ctx.enter_context(tc.tile_pool(name="ps_s", bufs=2, space="PSUM"))
        ps_o = ctx.enter_context(tc.tile_pool(name="ps_o", bufs=4, space="PSUM"))

        args = (nc, tc, const, big, work, small, stage, ps_s, ps_o,
                a_dram, v_dram, at_dram, vt_dram, wt_dram, b_dram, out_dram)
        if loop:
            with tc.For_i(0, loop):
                _kernel_body(*args)
        else:
            for _rep in range(repeat):
                _kernel_body(*args)

    nc.compile()
    _CACHE[key] = nc
    return nc


def _kernel_body(nc, tc, const, big, work, small, stage, ps_s, ps_o,
                 a_dram, v_dram, at_dram, vt_dram, wt_dram, b_dram, out_dram):
    # ---- load operands (bf16; transposed copies prepared on host) ----
    # Issue order = startup critical path: the first WV matmul needs all of
    # WT plus VT[0]; put those 8 tiles at the head of the DMA queue.
    # Two HWDGE rings (sync=SP, scalar=ACT) load in parallel. Priority:
    # WV needs WT+VT[0] first; pair 0 then needs AT[0] (scores), Vbf[0]
    # (fa) and Abf[0] (fv) before the later pairs' operands.
    WT = [big.tile([P, C], BF16, tag=f"WT{cc}", name=f"WT{cc}")
          for cc in range(ND)]
    for cc in range(ND):
        nc.sync.dma_start(WT[cc][:], wt_dram[cc * P:(cc + 1) * P, :])

    VT = [[big.tile([P, T], BF16, tag=f"VT{i}_{cc}", name=f"VT{i}_{cc}")
           for cc in range(ND)] for i in range(3)]
    AT = [[big.tile([P, T], BF16, tag=f"AT{j}_{cc}", name=f"AT{j}_{cc}")
           for cc in range(ND)] for j in range(3)]
    Abf = [[big.tile([P, C], BF16, tag=f"A{j}_{tb}", name=f"A{j}_{tb}")
            for tb in range(NT)] for j in range(3)]
    Vbf = [[big.tile([P, C], BF16, tag=f"V{i}_{tb}", name=f"V{i}_{tb}")
            for tb in range(NT)] for i in range(3)]

    for cc in range(ND):
        nc.scalar.dma_start(VT[0][cc][:], vt_dram[0][cc * P:(cc + 1) * P, :])

    b_sb = []
    for dc in range(ND):
        t_ = const.tile([P, 1], F32, tag=f"b{dc}", name=f"b{dc}")
        nc.sync.dma_start(t_[:], b_dram[dc])
        b_sb.append(t_)

    for cc in range(ND):
        nc.sync.dma_start(AT[0][cc][:], at_dram[0][cc * P:(cc + 1) * P, :])
    for tb in range(NT):
        nc.scalar.dma_start(Vbf[0][tb][:], v_dram[0][tb * P:(tb + 1) * P, :])
    for tb in range(NT):
        nc.sync.dma_start(Abf[0][tb][:], a_dram[0][tb * P:(tb + 1) * P, :])

    for i in range(1, 3):
        for cc in range(ND):
            nc.scalar.dma_start(VT[i][cc][:], vt_dram[i][cc * P:(cc + 1) * P, :])
    for j in range(1, 3):
        for cc in range(ND):
            nc.sync.dma_start(AT[j][cc][:], at_dram[j][cc * P:(cc + 1) * P, :])
    for i in range(1, 3):
        for tb in range(NT):
            nc.scalar.dma_start(Vbf[i][tb][:], v_dram[i][tb * P:(tb + 1) * P, :])
    for j in range(1, 3):
        for tb in range(NT):
            nc.sync.dma_start(Abf[j][tb][:], a_dram[j][tb * P:(tb + 1) * P, :])

    # ---- WV^T_i[d, t] = W^T @ V^T_i + b (bf16 out, bias folded in) ----
    WVT = [[big.tile([P, T], BF16, tag=f"WVT{i}_{dc}", name=f"WVT{i}_{dc}")
            for dc in range(ND)] for i in range(3)]
    for i in range(3):
        for dc in range(ND):
            po2 = [ps_o.tile([P, C], F32, tag="o", name="o") for _ in range(2)]
            for cc in range(ND):
                for th in range(2):
                    nc.tensor.matmul(po2[th][:], WT[cc][:, dc * P:(dc + 1) * P],
                                     VT[i][cc][:, th * C:(th + 1) * C],
                                     start=(cc == 0), stop=(cc == ND - 1))
            for th in range(2):
                nc.scalar.activation(WVT[i][dc][:, th * C:(th + 1) * C],
                                     po2[th][:], AF.Identity,
                                     bias=b_sb[dc][:], scale=1.0)

    # ---- main loop over the 9 attention pairs ----
    # Emission order is software-pipelined [scores_n | fv_{n-1} | fa_n]:
    # fv of the previous pair (whose P^T is long ready) fills the PE gap
    # while pair n's last exp/normalize drains, so fa_n never stalls.
    # Pt: row-normalized P, t-blocks side by side: Pt[t, tb*T + s]
    # PTb: P^T via DMA xbar transposes: PTb[s, (tb*NT+sc)*P + t];
    # parity double-buffered so pair n's transposes (emitted inside
    # scores_n, i.e. before fv_{n-1}) don't clobber what fv_{n-1} reads.
    Pt = work.tile([P, NT * T], BF16, tag="Pt", name="Pt")
    if _VARIANT in ("no_xpose", "mm_floor"):
        PTbs = [Pt, Pt]
    else:
        PTbs = [work.tile([P, NT * T], BF16, tag=f"PTb{par}",
                          name=f"PTb{par}") for par in range(2)]
    PTvs = [t.rearrange("p (q t) -> p q t", q=NT * NT) for t in PTbs]
    if _VARIANT == "mm_floor":
        nc.vector.memset(Pt[:], 0.5)

    pairs = [(i, j) for i in range(3) for j in range(3)]

    def emit_scores(n):
        i, j = pairs[n]
        PTv = PTvs[n % 2]
        for tb in range(NT):
            # one [128, 1024] score block = 2 PSUM banks; each matmul
            # stays within one bank.
            ps = ps_s.tile([P, T], F32, tag="s", name="s")
            if _SCORES_SHARE_LDW:
                for dc in range(ND):
                    for h in range(2):
                        nc.tensor.matmul(ps[:, h * C:(h + 1) * C],
                                         WVT[i][dc][:, tb * P:(tb + 1) * P],
                                         AT[j][dc][:, h * C:(h + 1) * C],
                                         start=(dc == 0), stop=(dc == ND - 1))
            else:
                for h in range(2):
                    for dc in range(ND):
                        nc.tensor.matmul(ps[:, h * C:(h + 1) * C],
                                         WVT[i][dc][:, tb * P:(tb + 1) * P],
                                         AT[j][dc][:, h * C:(h + 1) * C],
                                         start=(dc == 0), stop=(dc == ND - 1))
            if _VARIANT == "mm_floor":
                continue
            rsum = small.tile([P, 1], F32, tag=f"rs{tb}", name=f"rs{tb}")
            nc.scalar.activation(Pt[:, tb * T:(tb + 1) * T], ps[:], AF.Exp,
                                 scale=SCALE, accum_out=rsum[:])
            recip = small.tile([P, 1], F32, tag=f"rc{tb}", name=f"rc{tb}")
            nc.vector.reciprocal(recip[:], rsum[:])
            nc.vector.tensor_scalar_mul(Pt[:, tb * T:(tb + 1) * T],
                                        Pt[:, tb * T:(tb + 1) * T],
                                        recip[:])
            # transpose P in two halves, each issued as soon as its blocks
            # are normalized; the xbar runs under the remaining PE work
            if _VARIANT != "no_xpose" and tb in (NT // 2 - 1, NT - 1):
                half = tb // (NT // 2)
                qh, th_ = NT * NT // 2, NT * T // 2
                nc.scalar.dma_start_transpose(
                    PTv[:, half * qh:(half + 1) * qh, :],
                    Pt[:, half * th_:(half + 1) * th_])

    def emit_fa(n):
        # fa[j,i] = P^T @ V_i: P slices stationary, contraction over t
        i, j = pairs[n]
        for k in range(NT):
            po = ps_o.tile([P, C], F32, tag="o", name="o")
            for tb in range(NT):
                nc.tensor.matmul(po[:],
                                 Pt[:, tb * T + k * P: tb * T + (k + 1) * P],
                                 Vbf[i][tb][:],
                                 start=(tb == 0), stop=(tb == NT - 1))
            if _VARIANT == "mm_floor":
                continue
            st = stage.tile([P, C], F32, tag="st", name="st")
            nc.vector.tensor_copy(st[:], po[:])
            nc.sync.dma_start(out_dram[1, j, i, k * P:(k + 1) * P, :], st[:])

    def emit_fv(n):
        # fv[i,j] = P @ A_j: P^T slices stationary, contraction over s
        i, j = pairs[n]
        PTv = PTvs[n % 2]
        for k in range(NT):
            po = ps_o.tile([P, C], F32, tag="o", name="o")
            for sc in range(NT):
                nc.tensor.matmul(po[:], PTv[:, k * NT + sc, :],
                                 Abf[j][sc][:],
                                 start=(sc == 0), stop=(sc == NT - 1))
            if _VARIANT == "mm_floor":
                continue
            st = stage.tile([P, C], F32, tag="st", name="st")
            nc.scalar.activation(st[:], po[:], AF.Copy, bias=0.0, scale=1.0)
            nc.sync.dma_start(out_dram[0, i, j, k * P:(k + 1) * P, :], st[:])

    if _PIPELINED:
        for n in range(len(pairs)):
            emit_scores(n)
            if n > 0:
                emit_fv(n - 1)
            emit_fa(n)
        emit_fv(len(pairs) - 1)
    else:
        for n in range(len(pairs)):
            emit_scores(n)
            emit_fa(n)
            emit_fv(n)


def _prep_in_maps(a0, a1, a2, v0, v1, v2, W, b):
    bf = ml_dtypes.bfloat16
    a_bf = [np.asarray(x, dtype=np.float32).astype(bf) for x in (a0, a1, a2)]
    v_bf = [np.asarray(x, dtype=np.float32).astype(bf) for x in (v0, v1, v2)]
    wt_bf = np.ascontiguousarray(np.asarray(W, dtype=np.float32).astype(bf).T)
    b_r = np.ascontiguousarray(
        np.asarray(b, dtype=np.float32).reshape(ND, P, 1))
    in_maps = []
    for bi in range(B):
        m = {f"a{j}": np.ascontiguousarray(a_bf[j][bi]) for j in range(3)}
        m.update({f"v{i}": np.ascontiguousarray(v_bf[i][bi]) for i in range(3)})
        m.update({f"at{j}": np.ascontiguousarray(a_bf[j][bi].T)
                  for j in range(3)})
        m.update({f"vt{i}": np.ascontiguousarray(v_bf[i][bi].T)
                  for i in range(3)})
        m["WT"] = wt_bf
        m["bvec"] = b_r
        in_maps.append(m)
    return in_maps


def run(inputs, trace=False, tmpdir=None):
    """Build+run on 8 cores; returns (full_output, BassKernelResults)."""
    nc = _build()
    in_maps = _prep_in_maps(**inputs)
    res = run_bass_kernel_spmd(nc, in_maps, list(range(B)), trace=trace,
                               tmpdir=tmpdir)
    out = np.empty((2, 3, 3, B, T, C), dtype=np.float32)
    for bi in range(B):
        out[:, :, :, bi] = res.results[bi]["out"]
    return out, res


def kernel(a0, a1, a2, v0, v1, v2, W, b):
    out, _ = run(dict(a0=a0, a1=a1, a2=a2, v0=v0, v1=v1, v2=v2, W=W, b=b))
    return out

